# revision 16
# baseline (speedup 1.0000x reference)
# Trainium2 Bass kernel for single-query dot-product attention (decode step).
#
#   attn = softmax(q @ C^T)          q: (B, 1, H)  C: (B, S, H)
#   out  = tanh(attn @ C)
#   returns (out, attn)              B=32, S=4096, H=1024, fp32
#
# Sharding: batch-parallel, 4 batches per core across 8 NeuronCores.
#
# Per-core dataflow (per batch, single HBM pass over C):
#   - DMA C in [128, 2, 1024] chunks (partition = s mod 128, 1 MiB per dma)
#   - scores: DVE tensor_tensor_reduce computes prod_t = C_t * q_rep
#     (written as float32r for the later PE matmul) and accumulates
#     scores[:, t] = sum_h prod in the same pass
#   - softmax: DVE free-axis max, GPSIMD partition all-reduce (max),
#     ACT exp-with-accum (row sums), GPSIMD all-reduce (add), normalize
#   - out' = P^T @ prod on PE in fp32r (attn weights stationary, prod
#     streaming, PSUM accumulation over all 32 s-subtiles). Since
#     prod = C .* q, out' = out .* q; divide by q (DVE, 2-ULP approx
#     reciprocal) and tanh on ACT.
#   - attn transposed on PE so the HBM writeout is contiguous.
from contextlib import ExitStack

import numpy as np

import concourse.bass as bass
import concourse.bacc as bacc
import concourse.mybir as mybir
import concourse.bass_isa as bass_isa
import concourse.tile as tile
from concourse.bass_utils import run_bass_kernel_spmd
from concourse.masks import make_identity

B, S, H = 32, 4096, 1024
NCORES = 8
BPC = B // NCORES          # batches per core
NT = S // 128              # 32 s-subtiles of 128 rows per batch
TPD = 4                    # s-subtiles per DMA chunk (2 MiB per dma_start)
ND = NT // TPD             # dma chunks per batch

F32 = mybir.dt.float32
F32R = mybir.dt.float32r

CPOOL_BUFS = 3
PROD_BUFS = 56
WARM_EVERY = 2             # dummy PE matmul every N score tiles (keeps HAM hot)
PROD_DT = mybir.dt.bfloat16  # fp32r for higher precision, bf16 for SBUF depth

LAST_RESULTS = None        # test.py reads profiling info from here


def _build(tc, q_ap, c_ap, out_ap, attn_ap, repeat=1):
    nc = tc.nc
    ctx = ExitStack()
    with ctx:
        cpool = ctx.enter_context(tc.tile_pool(name="cpool", bufs=CPOOL_BUFS))
        prodp = ctx.enter_context(tc.tile_pool(name="prodp", bufs=PROD_BUFS))
        qpool = ctx.enter_context(tc.tile_pool(name="qpool", bufs=2))
        smallp = ctx.enter_context(tc.tile_pool(name="smallp", bufs=2))
        singles = ctx.enter_context(tc.tile_pool(name="singles", bufs=1))
        psump = ctx.enter_context(tc.tile_pool(name="psump", bufs=2, space="PSUM"))

        identity = singles.tile([128, 128], F32)
        make_identity(nc, identity)
        warm_psum = psump.tile([1, 1], F32, tag="warm", bufs=1)

        if repeat > 1:
            # timing amplification only: run the whole per-core program
            # `repeat` times inside a device-side loop
            loop = ctx.enter_context(tc.For_i(0, repeat, 1))

        for b in range(BPC):
            # q broadcast to all 128 partitions: [128, H]
            q_rep = qpool.tile([128, H], F32)
            q_src = bass.AP(
                tensor=q_ap.tensor, offset=b * H, ap=[[0, 128], [1, H]]
            )
            nc.gpsimd.dma_start(out=q_rep, in_=q_src)

            # 1/q for the final un-scaling (prod tiles carry a factor of q)
            q_inv = smallp.tile([1, H], F32, tag="q_inv")
            q_inv_scratch = smallp.tile([1, H], F32, tag="q_inv_scratch")
            nc.vector.reciprocal_approx_accurate(
                out=q_inv, in_=q_rep[0:1, :], scratch=q_inv_scratch
            )

            # C for this batch: s = t*128 + p  ->  [p, t, h]
            c_resh = c_ap[b].rearrange("(t p) h -> p t h", p=128)

            scores = smallp.tile([128, NT], F32, tag="scores")
            prods = []
            for j in range(ND):
                c_tile = cpool.tile([128, TPD, H], F32, tag="C")
                nc.sync.dma_start(
                    out=c_tile, in_=c_resh[:, j * TPD : (j + 1) * TPD, :]
                )
                for k in range(TPD):
                    t = j * TPD + k
                    prod = prodp.tile([128, H], PROD_DT, tag="prod")
                    # prod = C_t * q (rounded to fp32r for the PE matmul),
                    # scores[:, t] = sum_h prod  — one DVE pass
                    nc.vector.scalar_tensor_tensor(
                        out=prod,
                        in0=c_tile[:, k, :],
                        scalar=0.0,
                        in1=q_rep,
                        op0=mybir.AluOpType.bypass,
                        op1=mybir.AluOpType.mult,
                        accum_out=scores[:, t : t + 1],
                    )
                    prods.append(prod)
                    if t % WARM_EVERY == 1:
                        # tiny dependent matmul: keeps the PE HAM window
                        # busy through the scores phase so the real burst
                        # runs at 2.4 GHz
                        nc.tensor.matmul(
                            warm_psum,
                            lhsT=scores[:, t : t + 1],
                            rhs=scores[:, t : t + 1],
                            start=True,
                            stop=True,
                        )

            # global max over the whole [128, NT] score block
            mx = smallp.tile([128, 1], F32, tag="mx")
            nc.vector.reduce_max(mx, scores, axis=mybir.AxisListType.X)
            m_all = smallp.tile([128, 1], F32, tag="m_all")
            nc.gpsimd.partition_all_reduce(
                m_all, mx, channels=128, reduce_op=bass_isa.ReduceOp.max
            )
            negm = smallp.tile([128, 1], F32, tag="negm")
            nc.vector.tensor_scalar_mul(negm, m_all, -1.0)

            # E = exp(scores - max), z_col[p] = sum_t E[p, t]
            e_blk = smallp.tile([128, NT], F32, tag="e_blk")
            z_col = smallp.tile([128, 1], F32, tag="z_col")
            nc.scalar.activation(
                out=e_blk,
                in_=scores,
                func=mybir.ActivationFunctionType.Exp,
                bias=negm,
                scale=1.0,
                accum_out=z_col,
            )
            # fp32r copy of E: the output matmul runs on UNNORMALIZED
            # weights so it can start right after the exp; 1/Z is folded
            # into the final [1, H] scale instead.
            e_r = smallp.tile([128, NT], PROD_DT, tag="e_r")
            nc.vector.tensor_scalar_mul(e_r, e_blk, 1.0)

            # out'' = E^T @ prod accumulated over all 32 s-subtiles, fp32r.
            # t outer so prod tiles free in order for the next batch.
            u_psum = psump.tile([1, H], F32, tag="U", bufs=1)
            for t in range(NT):
                for n in range(H // 512):
                    nc.tensor.matmul(
                        u_psum[:, n * 512 : (n + 1) * 512],
                        lhsT=e_r[:, t : t + 1],
                        rhs=prods[t][:, n * 512 : (n + 1) * 512],
                        start=(t == 0),
                        stop=(t == NT - 1),
                    )

            # Z and attention normalization (off the PE critical path)
            z_all = smallp.tile([128, 1], F32, tag="z_all")
            nc.gpsimd.partition_all_reduce(
                z_all, z_col, channels=128, reduce_op=bass_isa.ReduceOp.add
            )
            r_all = smallp.tile([128, 1], F32, tag="r_all")
            nc.vector.reciprocal(r_all, z_all)
            p_attn = smallp.tile([128, NT], F32, tag="p_attn")
            nc.vector.tensor_scalar_mul(p_attn, e_blk, r_all)

            # attn writeout: transpose [128, NT] -> [NT, 128] so HBM rows are
            # contiguous per partition
            pt_psum = psump.tile([NT, 128], F32, tag="pt")
            nc.tensor.transpose(pt_psum, p_attn, identity)
            attn_sb = smallp.tile([NT, 128], F32, tag="attn_sb")
            nc.scalar.copy(attn_sb, pt_psum)
            nc.sync.dma_start(
                out=attn_ap[b].rearrange("q (t j) -> (q t) j", j=128), in_=attn_sb
            )

            # out = tanh(out'' * (1/Z) / q) — single DVE pass + ACT tanh
            u2_psum = psump.tile([1, H], F32, tag="U2", bufs=1)
            nc.vector.scalar_tensor_tensor(
                out=u2_psum,
                in0=u_psum,
                scalar=r_all[0:1, :],
                in1=q_inv,
                op0=mybir.AluOpType.mult,
                op1=mybir.AluOpType.mult,
            )
            out_sb = smallp.tile([1, H], F32, tag="out_sb")
            nc.scalar.activation(
                out=out_sb, in_=u2_psum, func=mybir.ActivationFunctionType.Tanh
            )
            nc.sync.dma_start(out=out_ap[b], in_=out_sb)


def kernel(
    output: np.ndarray,
    context: np.ndarray,
    _trace: bool = False,
    _repeat: int = 1,
):
    global LAST_RESULTS
    output = np.ascontiguousarray(np.asarray(output, dtype=np.float32))
    context = np.ascontiguousarray(np.asarray(context, dtype=np.float32))
    assert output.shape == (B, 1, H) and context.shape == (B, S, H)

    nc = bacc.Bacc(
        "TRN2",
        target_bir_lowering=False,
        debug=False,
        enable_asserts=False,
        num_devices=NCORES,
    )
    q_t = nc.dram_tensor("q", [BPC, 1, H], F32, kind="ExternalInput")
    c_t = nc.dram_tensor("c", [BPC, S, H], F32, kind="ExternalInput")
    out_t = nc.dram_tensor("out", [BPC, 1, H], F32, kind="ExternalOutput")
    attn_t = nc.dram_tensor("attn", [BPC, 1, S], F32, kind="ExternalOutput")

    with tile.TileContext(nc) as tc:
        _build(tc, q_t.ap(), c_t.ap(), out_t.ap(), attn_t.ap(), repeat=_repeat)
    nc.compile()

    in_maps = [
        {
            "q": output[i * BPC : (i + 1) * BPC],
            "c": context[i * BPC : (i + 1) * BPC],
        }
        for i in range(NCORES)
    ]
    res = run_bass_kernel_spmd(
        nc, in_maps, core_ids=list(range(NCORES)), trace=_trace
    )
    LAST_RESULTS = res
    out = np.concatenate([r["out"] for r in res.results], axis=0)
    attn = np.concatenate([r["attn"] for r in res.results], axis=0)
    return out, attn


if __name__ == "__main__":
    rng = np.random.default_rng(0)
    q = rng.standard_normal((B, 1, H), dtype=np.float32)
    c = rng.standard_normal((B, S, H), dtype=np.float32)
    o, a = kernel(q, c)
    print(o.shape, a.shape, float(np.abs(o).max()), float(a.sum(axis=-1).mean()))


# revision 17
# speedup vs baseline: 1.0368x; 1.0368x over previous
# Trainium2 Bass kernel for single-query dot-product attention (decode step).
#
#   attn = softmax(q @ C^T)          q: (B, 1, H)  C: (B, S, H)
#   out  = tanh(attn @ C)
#   returns (out, attn)              B=32, S=4096, H=1024, fp32
#
# Sharding: batch-parallel, 4 batches per core across 8 NeuronCores.
#
# Per-core dataflow (per batch, single HBM pass over C):
#   - DMA C in [128, 4, 1024] fp32 chunks (partition = s mod 128, 2 MiB/dma)
#   - scores: one DVE scalar_tensor_tensor per s-subtile computes
#     prod_t = C_t * q_rep (stored bf16 for the later PE matmul) and
#     accumulates scores[:, t] = sum_h C_t*q in fp32 in the same pass
#   - softmax: DVE free-axis max, GPSIMD partition all-reduce (max),
#     ACT exp-with-accum (row sums), GPSIMD all-reduce (add)
#   - out'' = E^T @ prod on PE (unnormalized exp weights stationary, prod
#     streaming, PSUM accumulation over all 32 s-subtiles; starts right
#     after the exp). Since prod = C .* q, out'' = out .* q * Z; the final
#     [1, H] scale folds 1/Z and 1/q (2-ULP approx reciprocal), then tanh.
#   - tiny score-dependent dummy matmuls pepper the scores phase to hold
#     the PE HAM clock-gate open so the real burst runs at 2.4 GHz
#   - attn = E/Z transposed on PE so the HBM writeout is contiguous.
from contextlib import ExitStack

import numpy as np

import concourse.bass as bass
import concourse.bacc as bacc
import concourse.mybir as mybir
import concourse.bass_isa as bass_isa
import concourse.tile as tile
from concourse.bass_utils import run_bass_kernel_spmd
from concourse.masks import make_identity

B, S, H = 32, 4096, 1024
NCORES = 8
BPC = B // NCORES          # batches per core
NT = S // 128              # 32 s-subtiles of 128 rows per batch
TPD = 4                    # s-subtiles per DMA chunk (2 MiB per dma_start)
ND = NT // TPD             # dma chunks per batch

F32 = mybir.dt.float32
F32R = mybir.dt.float32r

CPOOL_BUFS = 3
PROD_BUFS = 56
WARM_EVERY = 2             # dummy PE matmul every N score tiles (keeps HAM hot)
PROD_DT = mybir.dt.bfloat16  # fp32r for higher precision, bf16 for SBUF depth

LAST_RESULTS = None        # test.py reads profiling info from here


def _build(tc, q_ap, c_ap, out_ap, attn_ap, repeat=1):
    nc = tc.nc
    ctx = ExitStack()
    with ctx:
        cpool = ctx.enter_context(tc.tile_pool(name="cpool", bufs=CPOOL_BUFS))
        prodp = ctx.enter_context(tc.tile_pool(name="prodp", bufs=PROD_BUFS))
        qpool = ctx.enter_context(tc.tile_pool(name="qpool", bufs=2))
        smallp = ctx.enter_context(tc.tile_pool(name="smallp", bufs=2))
        singles = ctx.enter_context(tc.tile_pool(name="singles", bufs=1))
        psump = ctx.enter_context(tc.tile_pool(name="psump", bufs=2, space="PSUM"))

        identity = singles.tile([128, 128], F32)
        make_identity(nc, identity)
        warm_psum = psump.tile([1, 1], F32, tag="warm", bufs=1)

        if repeat > 1:
            # timing amplification only: run the whole per-core program
            # `repeat` times inside a device-side loop
            loop = ctx.enter_context(tc.For_i(0, repeat, 1))

        for b in range(BPC):
            # q broadcast to all 128 partitions: [128, H]
            q_rep = qpool.tile([128, H], F32)
            q_src = bass.AP(
                tensor=q_ap.tensor, offset=b * H, ap=[[0, 128], [1, H]]
            )
            nc.gpsimd.dma_start(out=q_rep, in_=q_src)

            # 1/q for the final un-scaling (prod tiles carry a factor of q)
            q_inv = smallp.tile([1, H], F32, tag="q_inv")
            q_inv_scratch = smallp.tile([1, H], F32, tag="q_inv_scratch")
            nc.vector.reciprocal_approx_accurate(
                out=q_inv, in_=q_rep[0:1, :], scratch=q_inv_scratch
            )

            # C for this batch: s = t*128 + p  ->  [p, t, h]
            c_resh = c_ap[b].rearrange("(t p) h -> p t h", p=128)

            scores = smallp.tile([128, NT], F32, tag="scores")
            prods = []
            for j in range(ND):
                c_tile = cpool.tile([128, TPD, H], F32, tag="C")
                nc.sync.dma_start(
                    out=c_tile, in_=c_resh[:, j * TPD : (j + 1) * TPD, :]
                )
                for k in range(TPD):
                    t = j * TPD + k
                    prod = prodp.tile([128, H], PROD_DT, tag="prod")
                    # prod = C_t * q (rounded to fp32r for the PE matmul),
                    # scores[:, t] = sum_h prod  — one DVE pass
                    nc.vector.scalar_tensor_tensor(
                        out=prod,
                        in0=c_tile[:, k, :],
                        scalar=0.0,
                        in1=q_rep,
                        op0=mybir.AluOpType.bypass,
                        op1=mybir.AluOpType.mult,
                        accum_out=scores[:, t : t + 1],
                    )
                    prods.append(prod)
                    if t % WARM_EVERY == 1:
                        # tiny dependent matmul: keeps the PE HAM window
                        # busy through the scores phase so the real burst
                        # runs at 2.4 GHz
                        nc.tensor.matmul(
                            warm_psum,
                            lhsT=scores[:, t : t + 1],
                            rhs=scores[:, t : t + 1],
                            start=True,
                            stop=True,
                        )

            # global max over the whole [128, NT] score block
            mx = smallp.tile([128, 1], F32, tag="mx")
            nc.vector.reduce_max(mx, scores, axis=mybir.AxisListType.X)
            m_all = smallp.tile([128, 1], F32, tag="m_all")
            nc.gpsimd.partition_all_reduce(
                m_all, mx, channels=128, reduce_op=bass_isa.ReduceOp.max
            )
            negm = smallp.tile([128, 1], F32, tag="negm")
            nc.vector.tensor_scalar_mul(negm, m_all, -1.0)

            # E = exp(scores - max), z_col[p] = sum_t E[p, t]
            e_blk = smallp.tile([128, NT], F32, tag="e_blk")
            z_col = smallp.tile([128, 1], F32, tag="z_col")
            nc.scalar.activation(
                out=e_blk,
                in_=scores,
                func=mybir.ActivationFunctionType.Exp,
                bias=negm,
                scale=1.0,
                accum_out=z_col,
            )
            # fp32r copy of E: the output matmul runs on UNNORMALIZED
            # weights so it can start right after the exp; 1/Z is folded
            # into the final [1, H] scale instead.
            e_r = smallp.tile([128, NT], PROD_DT, tag="e_r")
            nc.vector.tensor_scalar_mul(e_r, e_blk, 1.0)

            # out'' = E^T @ prod accumulated over all 32 s-subtiles, fp32r.
            # t outer so prod tiles free in order for the next batch.
            u_psum = psump.tile([1, H], F32, tag="U", bufs=1)
            for t in range(NT):
                for n in range(H // 512):
                    nc.tensor.matmul(
                        u_psum[:, n * 512 : (n + 1) * 512],
                        lhsT=e_r[:, t : t + 1],
                        rhs=prods[t][:, n * 512 : (n + 1) * 512],
                        start=(t == 0),
                        stop=(t == NT - 1),
                    )

            # Z and attention normalization (off the PE critical path)
            z_all = smallp.tile([128, 1], F32, tag="z_all")
            nc.gpsimd.partition_all_reduce(
                z_all, z_col, channels=128, reduce_op=bass_isa.ReduceOp.add
            )
            r_all = smallp.tile([128, 1], F32, tag="r_all")
            nc.vector.reciprocal(r_all, z_all)
            p_attn = smallp.tile([128, NT], F32, tag="p_attn")
            nc.vector.tensor_scalar_mul(p_attn, e_blk, r_all)

            # attn writeout: transpose [128, NT] -> [NT, 128] so HBM rows are
            # contiguous per partition
            pt_psum = psump.tile([NT, 128], F32, tag="pt")
            nc.tensor.transpose(pt_psum, p_attn, identity)
            attn_sb = smallp.tile([NT, 128], F32, tag="attn_sb")
            nc.scalar.copy(attn_sb, pt_psum)
            nc.sync.dma_start(
                out=attn_ap[b].rearrange("q (t j) -> (q t) j", j=128), in_=attn_sb
            )

            # out = tanh(out'' * (1/Z) / q) — single DVE pass + ACT tanh
            u2_psum = psump.tile([1, H], F32, tag="U2", bufs=1)
            nc.vector.scalar_tensor_tensor(
                out=u2_psum,
                in0=u_psum,
                scalar=r_all[0:1, :],
                in1=q_inv,
                op0=mybir.AluOpType.mult,
                op1=mybir.AluOpType.mult,
            )
            out_sb = smallp.tile([1, H], F32, tag="out_sb")
            nc.scalar.activation(
                out=out_sb, in_=u2_psum, func=mybir.ActivationFunctionType.Tanh
            )
            nc.sync.dma_start(out=out_ap[b], in_=out_sb)


def kernel(
    output: np.ndarray,
    context: np.ndarray,
    _trace: bool = False,
    _repeat: int = 1,
):
    global LAST_RESULTS
    output = np.ascontiguousarray(np.asarray(output, dtype=np.float32))
    context = np.ascontiguousarray(np.asarray(context, dtype=np.float32))
    assert output.shape == (B, 1, H) and context.shape == (B, S, H)

    nc = bacc.Bacc(
        "TRN2",
        target_bir_lowering=False,
        debug=False,
        enable_asserts=False,
        num_devices=NCORES,
    )
    q_t = nc.dram_tensor("q", [BPC, 1, H], F32, kind="ExternalInput")
    c_t = nc.dram_tensor("c", [BPC, S, H], F32, kind="ExternalInput")
    out_t = nc.dram_tensor("out", [BPC, 1, H], F32, kind="ExternalOutput")
    attn_t = nc.dram_tensor("attn", [BPC, 1, S], F32, kind="ExternalOutput")

    with tile.TileContext(nc) as tc:
        _build(tc, q_t.ap(), c_t.ap(), out_t.ap(), attn_t.ap(), repeat=_repeat)
    nc.compile()

    in_maps = [
        {
            "q": output[i * BPC : (i + 1) * BPC],
            "c": context[i * BPC : (i + 1) * BPC],
        }
        for i in range(NCORES)
    ]
    res = run_bass_kernel_spmd(
        nc, in_maps, core_ids=list(range(NCORES)), trace=_trace
    )
    LAST_RESULTS = res
    out = np.concatenate([r["out"] for r in res.results], axis=0)
    attn = np.concatenate([r["attn"] for r in res.results], axis=0)
    return out, attn


if __name__ == "__main__":
    rng = np.random.default_rng(0)
    q = rng.standard_normal((B, 1, H), dtype=np.float32)
    c = rng.standard_normal((B, S, H), dtype=np.float32)
    o, a = kernel(q, c)
    print(o.shape, a.shape, float(np.abs(o).max()), float(a.sum(axis=-1).mean()))
